# revision 9
# baseline (speedup 1.0000x reference)
"""Trainium2 Bass kernel for MQA sparse attention (nn_Attention_83356725281353).

Batch-parallel attention across 8 NeuronCores (4 batches each) with
head-sharded projection weights and collective exchanges:

  - wq is sharded 2 heads/core; every core computes q for ALL 128 (b,q)
    rows for its 2 heads (M=128 matmuls, full PE util), plus k_new/v_new
    (wk/wv replicated, also M=128). An AllToAll then routes, per core,
    all 16 heads of q (plus k_new/v_new) for that core's own 16 rows.
  - Attention (per-core, 4 batches, kv streamed in 2048 chunks) is
    unchanged from the tuned baseline: p^T = kt.T @ qT with kt
    stationary, exp via DVE-add + ACT, o accumulated with a ones column
    appended to v giving softmax denominators for free.
  - wo is sharded 256 output-dims/core; an AllGather of the per-core
    attention outputs oT lets every core compute ALL 128 rows for its
    256-dim slice with M=128 matmuls. Host concatenates dim slices.

This removes the 14 MB/core of replicated wq/wo traffic (the kernel is
HBM-bound at ~358 GB/s/core) and the 12.5%-PE-utilization M=16
projection matmuls of the all-replicated variant. Streams are split
across both HWDGE rings (sync: kT+bias, scalar: v+weights) to reach the
per-core HBM cap.

Self-contained: hardcodes all shapes; builds/compiles once per process
and runs via run_bass_kernel_spmd on cores 0-7.
"""

import numpy as np

B, Q, DIM, H, HD, KV = 32, 4, 2048, 16, 128, 8192
NCORES = 8
BPC = B // NCORES            # 4 batches per core
BQ = BPC * Q                 # 16 own (b,q) rows per core
RALL = B * Q                 # 128 global rows
ROWS = H * Q                 # 64 attention rows per batch
NPAIR = BPC // 2             # 2 batch-pairs per core
DT = 16                      # dim tiles (DIM/128)
KCH = 2048                   # kv chunk width
NCH = KV // KCH              # 4 chunks per batch
HPC = H // NCORES            # 2 heads per core
DSH = DIM // NCORES          # 256 output dims per core
PW = HPC * HD + 2 * HD       # 512 proj columns (2 q heads + kn + vn)
RG = [list(range(NCORES))]

_CACHE = {}


def _build():
    import concourse.bass as bass
    import concourse.tile as tile
    from concourse import bacc, mybir, masks

    f32 = mybir.dt.float32
    bf16 = mybir.dt.bfloat16

    nc = bacc.Bacc("TRN2", target_bir_lowering=False, debug=False,
                   num_devices=NCORES)

    xT = nc.dram_tensor("xT", [DIM, RALL], bf16, kind="ExternalInput").ap()
    # per-core: [wq heads 2c,2c+1 (scaled) | wk | wv]
    wproj = nc.dram_tensor("wproj", [DIM, PW], bf16, kind="ExternalInput").ap()
    bproj = nc.dram_tensor("bproj", [1, PW], bf16, kind="ExternalInput").ap()
    kT = nc.dram_tensor("kT", [BPC, HD, KV], bf16, kind="ExternalInput").ap()
    vv = nc.dram_tensor("vv", [BPC, 128, KV // 128, HD + 1], bf16,
                        kind="ExternalInput").ap()
    # bias[j, p, c, (n t r)]: kv = c*2048 + n*512 + t*128 + p, r = pair-row
    bias = nc.dram_tensor("bias", [NPAIR, 128, NCH, KCH], bf16,
                          kind="ExternalInput").ap()
    wo = nc.dram_tensor("wo", [H * HD, DSH], bf16, kind="ExternalInput").ap()
    bo = nc.dram_tensor("bo", [1, DSH], bf16, kind="ExternalInput").ap()
    ones = nc.dram_tensor("ones", [1, RALL], bf16, kind="ExternalInput").ap()
    sel = nc.dram_tensor("sel", [RALL, BQ], bf16, kind="ExternalInput").ap()
    out = nc.dram_tensor("out", [RALL, DSH], f32, kind="ExternalOutput").ap()

    with tile.TileContext(nc) as tc:
        _body(tc, nc, bass, mybir, masks, xT, wproj, bproj, kT, vv,
              bias, wo, bo, ones, sel, out)

    nc.compile()
    return nc


def _body(tc, nc, bass, mybir, masks, xT, wproj, bproj, kT, vv,
          bias, wo, bo, ones, sel, out):
    from contextlib import ExitStack

    f32 = mybir.dt.float32
    bf16 = mybir.dt.bfloat16
    EXP = mybir.ActivationFunctionType.Exp

    with ExitStack() as octx:
        const = octx.enter_context(tc.tile_pool(name="const", bufs=1))
        wpool = octx.enter_context(tc.tile_pool(name="w", bufs=DT))
        kpool = octx.enter_context(tc.tile_pool(name="kt", bufs=10))
        vpool = octx.enter_context(tc.tile_pool(name="vt", bufs=10))
        bpool = octx.enter_context(tc.tile_pool(name="bias", bufs=5))
        apool = octx.enter_context(tc.tile_pool(name="a", bufs=6))
        wopool = octx.enter_context(tc.tile_pool(name="wo", bufs=H))
        dram = octx.enter_context(tc.tile_pool(name="dram", bufs=1,
                                               space="DRAM"))

        ident_f = const.tile([128, 128], f32, tag="idf")
        ident_b = const.tile([128, 128], bf16, tag="idb")
        masks.make_identity(nc, ident_f[:])
        masks.make_identity(nc, ident_b[:])
        ones16 = const.tile([1, RALL], bf16, tag="ones16")
        nc.gpsimd.dma_start(ones16[:], ones)

        xT_sb = const.tile([128, DT * RALL], bf16, tag="xT")
        nc.gpsimd.dma_start(xT_sb[:].rearrange("p (t m) -> p t m", t=DT),
                            xT.rearrange("(t p) m -> p t m", p=128))
        bproj_sb = const.tile([1, PW], bf16, tag="bproj")
        nc.gpsimd.dma_start(bproj_sb[:], bproj)
        bo_sb = const.tile([1, DSH], bf16, tag="bo")
        nc.gpsimd.dma_start(bo_sb[:], bo)

        sel_sb = const.tile([RALL, BQ], bf16, tag="sel")
        nc.gpsimd.dma_start(sel_sb[:], sel)
        proj_sb = const.tile([128, PW], bf16, tag="proj")
        gq_sb = const.tile([128, NCORES * HPC * HD], bf16, tag="gq")
        knT_sb = const.tile([128, BQ], bf16, tag="knT")
        # qT layout: [e, (b, h, q)] col = b*64 + h*4 + q (p-matmul moving)
        qT_sb = const.tile([128, BPC * ROWS], bf16, tag="qT")
        vn_sb = const.tile([BQ, HD], bf16, tag="vn")
        # oT layout: [e=128, (h,b,q)] col = h*16 + b*4 + q
        oT_sb = const.tile([128, BPC * ROWS], bf16, tag="oT")
        ogs_sb = const.tile([128, NCORES * BPC * ROWS], bf16, tag="ogs")
        oh_sb = const.tile([128, H * RALL], bf16, tag="oh")

        qg_in = dram.tile([128, HPC * HD], bf16, tag="qgin")
        qg_out = dram.tile([NCORES, 128, HPC * HD], bf16, tag="qgout",
                           addr_space="Shared")
        og_in = dram.tile([128, BPC * ROWS], bf16, tag="ogin")
        og_out = dram.tile([NCORES, 128, BPC * ROWS], bf16, tag="ogout",
                           addr_space="Shared")

        # ---------------- Phase P: projections -----------------------------
        # proj[row, :] for ALL 128 global rows: [q_h0 | q_h1 | kn | vn]
        with (tc.tile_pool(name="qps", bufs=1, space="PSUM") as qps,
              tc.tile_pool(name="wps", bufs=2, space="PSUM") as wps,
              tc.tile_pool(name="ptr", bufs=1, space="PSUM") as ptr):
            # warm the PE while xT/wproj stream in
            for _ in range(6):
                d_ps = wps.tile([128, 128], f32, tag="warm")
                nc.tensor.matmul(d_ps[:], ident_b[:], ident_b[:],
                                 start=True, stop=True)

            proj_ps = qps.tile([128, PW], f32, tag="projps")
            for t in range(DT):
                w_t = wpool.tile([128, PW], bf16, tag="wtile")
                nc.gpsimd.dma_start(w_t[:], wproj[t * 128:(t + 1) * 128, :])
                nc.tensor.matmul(proj_ps[:], xT_sb[:, t * 128:(t + 1) * 128],
                                 w_t[:], start=(t == 0), stop=False)
            ones_r = ones16[0:1, :]
            nc.tensor.matmul(proj_ps[:], ones_r[0:1, 0:128], bproj_sb[0:1, :],
                             start=False, stop=True)
            nc.vector.tensor_copy(proj_sb[:], proj_ps[:])

            # ship the q channels of proj (untransposed) to every core;
            # selection + transpose happen in PE select-matmuls below
            nc.gpsimd.dma_start(qg_in[:], proj_sb[:, 0:HPC * HD])
            nc.gpsimd.collective_compute(
                "AllGather", mybir.AluOpType.bypass, replica_groups=RG,
                ins=[qg_in.opt()], outs=[qg_out.opt()])
            # kn/vn for my rows are local: select from proj_sb with S
            # knT[e, j] = sum_r proj[r, 256+e] * S[r, j]
            knsel_ps = ptr.tile([128, BQ], f32, tag="knsel")
            nc.tensor.matmul(knsel_ps[:], proj_sb[:, 2 * HD:3 * HD],
                             sel_sb[:], start=True, stop=True)
            nc.vector.tensor_copy(knT_sb[:, 0:BQ], knsel_ps[:])
            vn_ps = ptr.tile([BQ, HD], f32, tag="vnsel")
            nc.tensor.matmul(vn_ps[:], sel_sb[:], proj_sb[:, 3 * HD:4 * HD],
                             start=True, stop=True)
            nc.vector.tensor_copy(vn_sb[:], vn_ps[:])
            # keep PE warm across the collective wait
            for _ in range(10):
                d_ps = wps.tile([128, 128], f32, tag="warm")
                nc.tensor.matmul(d_ps[:], ident_b[:], ident_b[:],
                                 start=True, stop=True)

            nc.gpsimd.dma_start(
                gq_sb[:].rearrange("p (i c) -> p i c", i=NCORES),
                qg_out[:].rearrange("i p c -> p i c"))
            # select-transpose: qsel[e, (i,h2,b,q)] = G_i_h2.T @ S
            qsel_ps = ptr.tile([128, H * BQ], f32, tag="qsel")
            for i in range(NCORES):
                for h2 in range(HPC):
                    nc.tensor.matmul(
                        qsel_ps[:, (i * HPC + h2) * BQ:(i * HPC + h2 + 1) * BQ],
                        gq_sb[:, (i * HPC + h2) * HD:(i * HPC + h2 + 1) * HD],
                        sel_sb[:], start=True, stop=True)
            # qT_sb[e, b*64 + (2i+h2)*4 + q] = qsel[e, (i*2+h2)*16 + b*4 + q]
            qs4 = qsel_ps[:].rearrange("p (i h2 b q) -> p b i h2 q",
                                       i=NCORES, h2=HPC, b=BPC)
            qT4 = qT_sb[:].rearrange("p (b i h2 q) -> p b i h2 q",
                                     b=BPC, i=NCORES, h2=HPC)
            for h2 in range(HPC):
                nc.vector.tensor_copy(qT4[:, :, :, h2, :],
                                      qs4[:, :, :, h2, :])

        # ---------------- Phase A: attention, per batch-pair ---------------
        VW = HD + 1
        with (tc.tile_pool(name="pps", bufs=4, space="PSUM") as pps,
              tc.tile_pool(name="tps", bufs=2, space="PSUM") as tps,
              tc.tile_pool(name="ops", bufs=2, space="PSUM") as ops):
            wo_tiles = []
            for j in range(NPAIR):
                b0, b1 = 2 * j, 2 * j + 1
                o_ps = ops.tile([128, VW], f32, tag="o")
                for c in range(NCH):
                    it = j * NCH + c
                    if it < H // 2:
                        for hh in range(2):
                            w_t = wopool.tile([128, DSH], bf16, tag="wot")
                            nc.scalar.dma_start(
                                w_t[:],
                                wo[(2 * it + hh) * HD:(2 * it + hh + 1) * HD, :])
                            wo_tiles.append(w_t)
                    kt0 = kpool.tile([128, KCH], bf16, tag="kt")
                    nc.sync.dma_start(kt0[:], kT[b0][:, c * KCH:(c + 1) * KCH])
                    kt1 = kpool.tile([128, KCH], bf16, tag="kt")
                    nc.sync.dma_start(kt1[:], kT[b1][:, c * KCH:(c + 1) * KCH])
                    v0 = vpool.tile([128, 16 * VW], bf16, tag="vt")
                    nc.scalar.dma_start(
                        v0[:].rearrange("p (n e) -> p n e", n=16),
                        vv[b0][:, c * 16:(c + 1) * 16, :])
                    v1 = vpool.tile([128, 16 * VW], bf16, tag="vt")
                    nc.scalar.dma_start(
                        v1[:].rearrange("p (n e) -> p n e", n=16),
                        vv[b1][:, c * 16:(c + 1) * 16, :])
                    bias_sb = bpool.tile([128, KCH], bf16, tag="bias")
                    nc.sync.dma_start(bias_sb[:], bias[j][:, c, :])
                    if c == NCH - 1:
                        nc.vector.tensor_copy(kt0[:, KCH - 4:KCH],
                                              knT_sb[:, b0 * 4:b0 * 4 + 4])
                        nc.vector.tensor_copy(kt1[:, KCH - 4:KCH],
                                              knT_sb[:, b1 * 4:b1 * 4 + 4])
                        nc.gpsimd.dma_start(
                            v0[124:128, 15 * VW:15 * VW + HD],
                            vn_sb[b0 * 4:b0 * 4 + 4, :])
                        nc.gpsimd.dma_start(
                            v1[124:128, 15 * VW:15 * VW + HD],
                            vn_sb[b1 * 4:b1 * 4 + 4, :])
                    for n in range(4):
                        p_ps = pps.tile([128, 512], f32, tag="p")
                        for t in range(4):
                            ko = (n * 4 + t) * 128
                            nc.tensor.matmul(
                                p_ps[:, t * 128:t * 128 + ROWS],
                                kt0[:, ko:ko + 128],
                                qT_sb[:, b0 * ROWS:(b0 + 1) * ROWS],
                                start=True, stop=True)
                            nc.tensor.matmul(
                                p_ps[:, t * 128 + ROWS:(t + 1) * 128],
                                kt1[:, ko:ko + 128],
                                qT_sb[:, b1 * ROWS:(b1 + 1) * ROWS],
                                start=True, stop=True)
                        e_sb = apool.tile([128, 512], f32, tag="e")
                        nc.vector.tensor_tensor(
                            e_sb[:], p_ps[:], bias_sb[:, n * 512:(n + 1) * 512],
                            op=mybir.AluOpType.add)
                        a_bf = apool.tile([128, 512], bf16, tag="abf")
                        nc.scalar.activation(a_bf[:], e_sb[:], EXP)
                        for t in range(4):
                            kvt = c * 16 + n * 4 + t
                            first, last = (kvt == 0), (kvt == 63)
                            vo = (n * 4 + t) * VW
                            nc.tensor.matmul(
                                o_ps[0:ROWS, :],
                                a_bf[:, t * 128:t * 128 + ROWS],
                                v0[:, vo:vo + VW], start=first, stop=last)
                            nc.tensor.matmul(
                                o_ps[ROWS:128, :],
                                a_bf[:, t * 128 + ROWS:(t + 1) * 128],
                                v1[:, vo:vo + VW], start=first, stop=last,
                                tile_position=(0, 64))
                        if j == NPAIR - 1 and c == NCH - 1:
                            # keep the PE activity window busy through the
                            # DVE/ACT-paced drain of the last chunk so the
                            # HAM clock gate stays at 2.4 GHz for phase O
                            for _ in range(2):
                                d_ps = pps.tile([128, 512], f32, tag="p")
                                nc.tensor.matmul(d_ps[:, :], ident_b[:],
                                                 bias_sb[:, 0:512],
                                                 start=True, stop=True)
                _finalize_pair(tc, nc, mybir, apool, tps, j, o_ps, oT_sb,
                               ident_f)
                if j == NPAIR - 1:
                    for _ in range(3):
                        d_ps = pps.tile([128, 512], f32, tag="p")
                        nc.tensor.matmul(d_ps[:, :], ident_b[:],
                                         bias_sb[:, 0:512],
                                         start=True, stop=True)

        # ---------------- Phase O: gather oT, output projection -------------
        with tc.tile_pool(name="outps", bufs=2, space="PSUM") as outps:
            nc.gpsimd.dma_start(og_in[:], oT_sb[:])
            nc.gpsimd.collective_compute(
                "AllGather", mybir.AluOpType.bypass, replica_groups=RG,
                ins=[og_in.opt()], outs=[og_out.opt()])
            nc.gpsimd.dma_start(
                ogs_sb[:].rearrange("p (i c) -> p i c", i=NCORES),
                og_out[:].rearrange("i p c -> p i c"))
            # keep PE warm across the collective wait
            for _ in range(8):
                d_ps = outps.tile([128, 128], f32, tag="warm")
                nc.tensor.matmul(d_ps[:], ident_b[:], ident_b[:],
                                 start=True, stop=True)
            # oh_sb[e, h*128 + i*16 + r] = ogs_sb[e, i*256 + h*16 + r]
            nc.vector.tensor_copy(
                oh_sb[:].rearrange("p (h i r) -> p h i r", h=H, i=NCORES),
                ogs_sb[:].rearrange("p (i h r) -> p h i r", i=NCORES, h=H))
            out_ps = outps.tile([RALL, DSH], f32, tag="out")
            for h in range(H):
                nc.tensor.matmul(out_ps[:], oh_sb[:, h * 128:(h + 1) * 128],
                                 wo_tiles[h][:], start=(h == 0), stop=False)
            ones_r = ones16[0:1, :]
            nc.tensor.matmul(out_ps[:], ones_r[0:1, 0:RALL], bo_sb[0:1, :],
                             start=False, stop=True)
            out_sb = const.tile([RALL, DSH], f32, tag="osb")
            nc.vector.tensor_copy(out_sb[:], out_ps[:])
            nc.scalar.dma_start(out, out_sb[:])


def _finalize_pair(tc, nc, mybir, apool, tps, j, o_ps, oT_sb, ident_f):
    f32 = mybir.dt.float32
    recip = apool.tile([128, 1], f32, tag="recip")
    nc.vector.reciprocal(recip[:], o_ps[:, HD:HD + 1])
    o_sb = apool.tile([128, HD], f32, tag="osb")
    nc.vector.tensor_scalar_mul(o_sb[:], o_ps[:, 0:HD], recip[:])
    tr = tps.tile([128, 128], f32, tag="tr")
    nc.tensor.transpose(tr[:], o_sb[:], ident_f[:])
    oT_4d = oT_sb[:].rearrange("p (h b q) -> p h b q", h=H, b=BPC)
    for b2 in range(2):
        nc.vector.tensor_copy(
            oT_4d[:, :, 2 * j + b2, :],
            tr[:, b2 * ROWS:(b2 + 1) * ROWS].rearrange(
                "p (h q) -> p h q", h=H))


def _get_nc():
    if "nc" not in _CACHE:
        _CACHE["nc"] = _build()
    return _CACHE["nc"]


def kernel(x, attn_bias, cache_k, cache_v, wq, bq, wk, bk, wv, bv, wo, bo):
    import ml_dtypes
    from concourse.bass_utils import run_bass_kernel_spmd

    nc = _get_nc()
    scale = np.float32(1.0 / np.sqrt(HD))
    bf = ml_dtypes.bfloat16

    x = np.asarray(x, np.float32)
    xT_full = np.ascontiguousarray(x.reshape(RALL, DIM).T).astype(bf)
    wq_s = np.asarray(wq, np.float32) * scale          # [DIM, H, HD]
    bq_s = np.asarray(bq, np.float32) * scale          # [H, HD]
    wk_f = np.asarray(wk, np.float32)
    wv_f = np.asarray(wv, np.float32)
    bk_f = np.asarray(bk, np.float32)
    bv_f = np.asarray(bv, np.float32)
    kTh = np.ascontiguousarray(
        np.roll(np.asarray(cache_k, np.float32), -Q, axis=1)
        .transpose(0, 2, 1)).astype(bf)
    vr0 = np.roll(np.asarray(cache_v, np.float32), -Q, axis=1)
    # [B, KV, HD] -> [B, 128, KV/128, HD+1]: per-partition-contiguous runs,
    # last column = 1.0 so the o-matmul accumulates softmax denominators
    vrh4 = vr0.reshape(B, KV // 128, 128, HD).transpose(0, 2, 1, 3)
    vrh = np.ones((B, 128, KV // 128, HD + 1), np.float32)
    vrh[..., :HD] = vrh4
    vrh = np.ascontiguousarray(vrh).astype(bf)
    # bias -> [pair, p, c, (n t r)] with kv = c*2048 + n*512 + t*128 + p
    ab = np.asarray(attn_bias, np.float32).reshape(B // 2, 2, ROWS, KV)
    abP = ab.transpose(0, 3, 1, 2).reshape(B // 2, KV, 2 * ROWS)
    biasP = np.ascontiguousarray(
        abP.reshape(B // 2, NCH, 4, 4, 128, 2 * ROWS)
        .transpose(0, 4, 1, 2, 3, 5)
        .reshape(B // 2, 128, NCH, KCH)).astype(bf)
    wo_f = np.asarray(wo, np.float32).reshape(H * HD, DIM)
    bo_f = np.asarray(bo, np.float32)

    selm = np.eye(RALL, dtype=np.float32).astype(bf)   # [RALL, RALL]
    in_maps = []
    for c in range(NCORES):
        wproj = np.concatenate(
            [wq_s[:, 2 * c:2 * c + 2, :].reshape(DIM, HPC * HD),
             wk_f, wv_f], axis=1)
        bproj = np.concatenate(
            [bq_s[2 * c:2 * c + 2].reshape(HPC * HD), bk_f, bv_f])
        in_maps.append({
            "xT": xT_full,
            "wproj": np.ascontiguousarray(wproj).astype(bf),
            "bproj": np.ascontiguousarray(bproj.reshape(1, PW)).astype(bf),
            "kT": np.ascontiguousarray(kTh[c * BPC:(c + 1) * BPC]),
            "vv": np.ascontiguousarray(vrh[c * BPC:(c + 1) * BPC]),
            "bias": np.ascontiguousarray(biasP[NPAIR * c:NPAIR * (c + 1)]),
            "wo": np.ascontiguousarray(
                wo_f[:, c * DSH:(c + 1) * DSH]).astype(bf),
            "bo": np.ascontiguousarray(
                bo_f[c * DSH:(c + 1) * DSH].reshape(1, DSH)).astype(bf),
            "ones": np.ones((1, RALL), bf),
            "sel": np.ascontiguousarray(selm[:, c * BQ:(c + 1) * BQ]),
        })

    res = run_bass_kernel_spmd(nc, in_maps, core_ids=list(range(NCORES)))
    _CACHE["last_result"] = res
    outs = [res.results[c]["out"] for c in range(NCORES)]
    full = np.concatenate(outs, axis=1)                # [128, DIM]
    return full.reshape(B, Q, DIM).astype(np.float32)


# revision 10
# speedup vs baseline: 1.0199x; 1.0199x over previous
"""Trainium2 Bass kernel for MQA sparse attention (nn_Attention_83356725281353).

Batch-parallel attention across 8 NeuronCores (4 batches each) with
head-sharded projection weights and collective exchanges:

  - wq is sharded 2 heads/core; every core computes q for ALL 128 (b,q)
    rows for its 2 heads (M=128 matmuls, full PE util), plus k_new/v_new
    (wk/wv replicated, also M=128). An AllToAll then routes, per core,
    all 16 heads of q (plus k_new/v_new) for that core's own 16 rows.
  - Attention (per-core, 4 batches, kv streamed in 2048 chunks) is
    unchanged from the tuned baseline: p^T = kt.T @ qT with kt
    stationary, exp via DVE-add + ACT, o accumulated with a ones column
    appended to v giving softmax denominators for free.
  - wo is sharded 256 output-dims/core; an AllGather of the per-core
    attention outputs oT lets every core compute ALL 128 rows for its
    256-dim slice with M=128 matmuls. Host concatenates dim slices.

This removes the 14 MB/core of replicated wq/wo traffic (the kernel is
HBM-bound at ~358 GB/s/core) and the 12.5%-PE-utilization M=16
projection matmuls of the all-replicated variant. Streams are split
across both HWDGE rings (sync: kT+bias, scalar: v+weights) to reach the
per-core HBM cap.

Self-contained: hardcodes all shapes; builds/compiles once per process
and runs via run_bass_kernel_spmd on cores 0-7.
"""

import numpy as np

B, Q, DIM, H, HD, KV = 32, 4, 2048, 16, 128, 8192
NCORES = 8
BPC = B // NCORES            # 4 batches per core
BQ = BPC * Q                 # 16 own (b,q) rows per core
RALL = B * Q                 # 128 global rows
ROWS = H * Q                 # 64 attention rows per batch
NPAIR = BPC // 2             # 2 batch-pairs per core
DT = 16                      # dim tiles (DIM/128)
KCH = 2048                   # kv chunk width
NCH = KV // KCH              # 4 chunks per batch
HPC = H // NCORES            # 2 heads per core
DSH = DIM // NCORES          # 256 output dims per core
PW = HPC * HD + 2 * HD       # 512 proj columns (2 q heads + kn + vn)
RG = [list(range(NCORES))]

_CACHE = {}


def _build():
    import concourse.bass as bass
    import concourse.tile as tile
    from concourse import bacc, mybir, masks

    f32 = mybir.dt.float32
    bf16 = mybir.dt.bfloat16

    nc = bacc.Bacc("TRN2", target_bir_lowering=False, debug=False,
                   num_devices=NCORES)

    xT = nc.dram_tensor("xT", [DIM, RALL], bf16, kind="ExternalInput").ap()
    # per-core: [wq heads 2c,2c+1 (scaled) | wk | wv]
    wproj = nc.dram_tensor("wproj", [DIM, PW], bf16, kind="ExternalInput").ap()
    bproj = nc.dram_tensor("bproj", [1, PW], bf16, kind="ExternalInput").ap()
    kT = nc.dram_tensor("kT", [BPC, HD, KV], bf16, kind="ExternalInput").ap()
    vv = nc.dram_tensor("vv", [BPC, 128, KV // 128, HD + 1], bf16,
                        kind="ExternalInput").ap()
    # bias[j, p, c, (n t r)]: kv = c*2048 + n*512 + t*128 + p, r = pair-row
    bias = nc.dram_tensor("bias", [NPAIR, 128, NCH, KCH], bf16,
                          kind="ExternalInput").ap()
    wo = nc.dram_tensor("wo", [H * HD, DSH], bf16, kind="ExternalInput").ap()
    bo = nc.dram_tensor("bo", [1, DSH], bf16, kind="ExternalInput").ap()
    ones = nc.dram_tensor("ones", [1, RALL], bf16, kind="ExternalInput").ap()
    sel = nc.dram_tensor("sel", [RALL, BQ], bf16, kind="ExternalInput").ap()
    out = nc.dram_tensor("out", [RALL, DSH], f32, kind="ExternalOutput").ap()

    with tile.TileContext(nc) as tc:
        _body(tc, nc, bass, mybir, masks, xT, wproj, bproj, kT, vv,
              bias, wo, bo, ones, sel, out)

    nc.compile()
    return nc


def _body(tc, nc, bass, mybir, masks, xT, wproj, bproj, kT, vv,
          bias, wo, bo, ones, sel, out):
    from contextlib import ExitStack

    f32 = mybir.dt.float32
    bf16 = mybir.dt.bfloat16
    EXP = mybir.ActivationFunctionType.Exp

    with ExitStack() as octx:
        const = octx.enter_context(tc.tile_pool(name="const", bufs=1))
        wpool = octx.enter_context(tc.tile_pool(name="w", bufs=DT))
        kpool = octx.enter_context(tc.tile_pool(name="kt", bufs=12))
        vpool = octx.enter_context(tc.tile_pool(name="vt", bufs=12))
        bpool = octx.enter_context(tc.tile_pool(name="bias", bufs=6))
        apool = octx.enter_context(tc.tile_pool(name="a", bufs=6))
        wopool = octx.enter_context(tc.tile_pool(name="wo", bufs=H))
        dram = octx.enter_context(tc.tile_pool(name="dram", bufs=1,
                                               space="DRAM"))

        ident_f = const.tile([128, 128], f32, tag="idf")
        ident_b = const.tile([128, 128], bf16, tag="idb")
        masks.make_identity(nc, ident_f[:])
        masks.make_identity(nc, ident_b[:])
        ones16 = const.tile([1, RALL], bf16, tag="ones16")

        xT_sb = const.tile([128, DT * RALL], bf16, tag="xT")
        bproj_sb = const.tile([1, PW], bf16, tag="bproj")
        bo_sb = const.tile([1, DSH], bf16, tag="bo")
        sel_sb = const.tile([RALL, BQ], bf16, tag="sel")
        with tc.high_priority():
            nc.scalar.dma_start(ones16[:], ones)
            nc.scalar.dma_start(
                xT_sb[:].rearrange("p (t m) -> p t m", t=DT),
                xT.rearrange("(t p) m -> p t m", p=128))
            nc.scalar.dma_start(bproj_sb[:], bproj)
            nc.scalar.dma_start(bo_sb[:], bo)
            nc.scalar.dma_start(sel_sb[:], sel)
        proj_sb = const.tile([128, PW], bf16, tag="proj")
        gq_sb = const.tile([128, NCORES * HPC * HD], bf16, tag="gq")
        knT_sb = const.tile([128, BQ], bf16, tag="knT")
        # qT layout: [e, (b, h, q)] col = b*64 + h*4 + q (p-matmul moving)
        qT_sb = const.tile([128, BPC * ROWS], bf16, tag="qT")
        vn_sb = const.tile([BQ, HD], bf16, tag="vn")
        # oT layout: [e=128, (h,b,q)] col = h*16 + b*4 + q
        oT_sb = const.tile([128, BPC * ROWS], bf16, tag="oT")
        ogs_sb = const.tile([128, NCORES * BPC * ROWS], bf16, tag="ogs")
        oh_sb = const.tile([128, H * RALL], bf16, tag="oh")

        qg_in = dram.tile([128, HPC * HD], bf16, tag="qgin")
        qg_out = dram.tile([NCORES, 128, HPC * HD], bf16, tag="qgout",
                           addr_space="Shared")
        og_in = dram.tile([128, BPC * ROWS], bf16, tag="ogin")
        og_out = dram.tile([NCORES, 128, BPC * ROWS], bf16, tag="ogout",
                           addr_space="Shared")

        # ---------------- Phase P: projections -----------------------------
        # proj[row, :] for ALL 128 global rows: [q_h0 | q_h1 | kn | vn]
        with (tc.tile_pool(name="qps", bufs=1, space="PSUM") as qps,
              tc.tile_pool(name="wps", bufs=2, space="PSUM") as wps,
              tc.tile_pool(name="ptr", bufs=1, space="PSUM") as ptr):
            # warm the PE while xT/wproj stream in
            for _ in range(6):
                d_ps = wps.tile([128, 128], f32, tag="warm")
                nc.tensor.matmul(d_ps[:], ident_b[:], ident_b[:],
                                 start=True, stop=True)

            w_tiles = []
            with tc.high_priority():
                for t in range(DT):
                    w_t = wpool.tile([128, PW], bf16, tag="wtile")
                    nc.scalar.dma_start(w_t[:],
                                        wproj[t * 128:(t + 1) * 128, :])
                    w_tiles.append(w_t)
            proj_ps = qps.tile([128, PW], f32, tag="projps")
            for t in range(DT):
                nc.tensor.matmul(proj_ps[:], xT_sb[:, t * 128:(t + 1) * 128],
                                 w_tiles[t][:], start=(t == 0), stop=False)
            ones_r = ones16[0:1, :]
            nc.tensor.matmul(proj_ps[:], ones_r[0:1, 0:128], bproj_sb[0:1, :],
                             start=False, stop=True)
            nc.vector.tensor_copy(proj_sb[:], proj_ps[:])

            # ship the q channels of proj (untransposed) to every core;
            # selection + transpose happen in PE select-matmuls below
            nc.gpsimd.dma_start(qg_in[:], proj_sb[:, 0:HPC * HD])
            nc.gpsimd.collective_compute(
                "AllGather", mybir.AluOpType.bypass, replica_groups=RG,
                ins=[qg_in.opt()], outs=[qg_out.opt()])
            # kn/vn for my rows are local: select from proj_sb with S
            # knT[e, j] = sum_r proj[r, 256+e] * S[r, j]
            knsel_ps = ptr.tile([128, BQ], f32, tag="knsel")
            nc.tensor.matmul(knsel_ps[:], proj_sb[:, 2 * HD:3 * HD],
                             sel_sb[:], start=True, stop=True)
            nc.vector.tensor_copy(knT_sb[:, 0:BQ], knsel_ps[:])
            vn_ps = ptr.tile([BQ, HD], f32, tag="vnsel")
            nc.tensor.matmul(vn_ps[:], sel_sb[:], proj_sb[:, 3 * HD:4 * HD],
                             start=True, stop=True)
            nc.vector.tensor_copy(vn_sb[:], vn_ps[:])
            # keep PE warm across the collective wait
            for _ in range(10):
                d_ps = wps.tile([128, 128], f32, tag="warm")
                nc.tensor.matmul(d_ps[:], ident_b[:], ident_b[:],
                                 start=True, stop=True)

            nc.gpsimd.dma_start(
                gq_sb[:].rearrange("p (i c) -> p i c", i=NCORES),
                qg_out[:].rearrange("i p c -> p i c"))
            # select-transpose: qsel[e, (i,h2,b,q)] = G_i_h2.T @ S
            qsel_ps = ptr.tile([128, H * BQ], f32, tag="qsel")
            for i in range(NCORES):
                for h2 in range(HPC):
                    nc.tensor.matmul(
                        qsel_ps[:, (i * HPC + h2) * BQ:(i * HPC + h2 + 1) * BQ],
                        gq_sb[:, (i * HPC + h2) * HD:(i * HPC + h2 + 1) * HD],
                        sel_sb[:], start=True, stop=True)
            # qT_sb[e, b*64 + (2i+h2)*4 + q] = qsel[e, (i*2+h2)*16 + b*4 + q]
            qs4 = qsel_ps[:].rearrange("p (i h2 b q) -> p b i h2 q",
                                       i=NCORES, h2=HPC, b=BPC)
            qT4 = qT_sb[:].rearrange("p (b i h2 q) -> p b i h2 q",
                                     b=BPC, i=NCORES, h2=HPC)
            for h2 in range(HPC):
                nc.vector.tensor_copy(qT4[:, :, :, h2, :],
                                      qs4[:, :, :, h2, :])

        # ---------------- Phase A: attention, per batch-pair ---------------
        VW = HD + 1
        with (tc.tile_pool(name="pps", bufs=4, space="PSUM") as pps,
              tc.tile_pool(name="tps", bufs=2, space="PSUM") as tps,
              tc.tile_pool(name="ops", bufs=2, space="PSUM") as ops):
            wo_tiles = []
            for j in range(NPAIR):
                b0, b1 = 2 * j, 2 * j + 1
                o_ps = ops.tile([128, VW], f32, tag="o")
                for c in range(NCH):
                    it = j * NCH + c
                    if it < H // 2:
                        for hh in range(2):
                            w_t = wopool.tile([128, DSH], bf16, tag="wot")
                            nc.scalar.dma_start(
                                w_t[:],
                                wo[(2 * it + hh) * HD:(2 * it + hh + 1) * HD, :])
                            wo_tiles.append(w_t)
                    kt0 = kpool.tile([128, KCH], bf16, tag="kt")
                    nc.sync.dma_start(kt0[:], kT[b0][:, c * KCH:(c + 1) * KCH])
                    kt1 = kpool.tile([128, KCH], bf16, tag="kt")
                    nc.sync.dma_start(kt1[:], kT[b1][:, c * KCH:(c + 1) * KCH])
                    v0 = vpool.tile([128, 16 * VW], bf16, tag="vt")
                    nc.scalar.dma_start(
                        v0[:].rearrange("p (n e) -> p n e", n=16),
                        vv[b0][:, c * 16:(c + 1) * 16, :])
                    v1 = vpool.tile([128, 16 * VW], bf16, tag="vt")
                    nc.scalar.dma_start(
                        v1[:].rearrange("p (n e) -> p n e", n=16),
                        vv[b1][:, c * 16:(c + 1) * 16, :])
                    bias_sb = bpool.tile([128, KCH], bf16, tag="bias")
                    nc.sync.dma_start(bias_sb[:], bias[j][:, c, :])
                    d_ps = pps.tile([128, 512], f32, tag="p")
                    nc.tensor.matmul(d_ps[:, 0:128], kt0[:, 0:128],
                                     ident_b[:], start=True, stop=True)
                    if c == NCH - 1:
                        nc.vector.tensor_copy(kt0[:, KCH - 4:KCH],
                                              knT_sb[:, b0 * 4:b0 * 4 + 4])
                        nc.vector.tensor_copy(kt1[:, KCH - 4:KCH],
                                              knT_sb[:, b1 * 4:b1 * 4 + 4])
                        nc.gpsimd.dma_start(
                            v0[124:128, 15 * VW:15 * VW + HD],
                            vn_sb[b0 * 4:b0 * 4 + 4, :])
                        nc.gpsimd.dma_start(
                            v1[124:128, 15 * VW:15 * VW + HD],
                            vn_sb[b1 * 4:b1 * 4 + 4, :])
                    for n in range(4):
                        p_ps = pps.tile([128, 512], f32, tag="p")
                        for t in range(4):
                            ko = (n * 4 + t) * 128
                            nc.tensor.matmul(
                                p_ps[:, t * 128:t * 128 + ROWS],
                                kt0[:, ko:ko + 128],
                                qT_sb[:, b0 * ROWS:(b0 + 1) * ROWS],
                                start=True, stop=True)
                            nc.tensor.matmul(
                                p_ps[:, t * 128 + ROWS:(t + 1) * 128],
                                kt1[:, ko:ko + 128],
                                qT_sb[:, b1 * ROWS:(b1 + 1) * ROWS],
                                start=True, stop=True)
                        e_sb = apool.tile([128, 512], f32, tag="e")
                        nc.vector.tensor_tensor(
                            e_sb[:], p_ps[:], bias_sb[:, n * 512:(n + 1) * 512],
                            op=mybir.AluOpType.add)
                        a_bf = apool.tile([128, 512], bf16, tag="abf")
                        nc.scalar.activation(a_bf[:], e_sb[:], EXP)
                        for t in range(4):
                            kvt = c * 16 + n * 4 + t
                            first, last = (kvt == 0), (kvt == 63)
                            vo = (n * 4 + t) * VW
                            nc.tensor.matmul(
                                o_ps[0:ROWS, :],
                                a_bf[:, t * 128:t * 128 + ROWS],
                                v0[:, vo:vo + VW], start=first, stop=last)
                            nc.tensor.matmul(
                                o_ps[ROWS:128, :],
                                a_bf[:, t * 128 + ROWS:(t + 1) * 128],
                                v1[:, vo:vo + VW], start=first, stop=last,
                                tile_position=(0, 64))
                        if j == NPAIR - 1 and c == NCH - 1:
                            # keep the PE activity window busy through the
                            # DVE/ACT-paced drain of the last chunk so the
                            # HAM clock gate stays at 2.4 GHz for phase O
                            for _ in range(2):
                                d_ps = pps.tile([128, 512], f32, tag="p")
                                nc.tensor.matmul(d_ps[:, :], ident_b[:],
                                                 bias_sb[:, 0:512],
                                                 start=True, stop=True)
                _finalize_pair(tc, nc, mybir, apool, tps, j, o_ps, oT_sb,
                               ident_f)
                if j == NPAIR - 1:
                    for _ in range(3):
                        d_ps = pps.tile([128, 512], f32, tag="p")
                        nc.tensor.matmul(d_ps[:, :], ident_b[:],
                                         bias_sb[:, 0:512],
                                         start=True, stop=True)

        # ---------------- Phase O: gather oT, output projection -------------
        with tc.tile_pool(name="outps", bufs=2, space="PSUM") as outps:
            nc.gpsimd.dma_start(og_in[:], oT_sb[:])
            nc.gpsimd.collective_compute(
                "AllGather", mybir.AluOpType.bypass, replica_groups=RG,
                ins=[og_in.opt()], outs=[og_out.opt()])
            nc.gpsimd.dma_start(
                ogs_sb[:].rearrange("p (i c) -> p i c", i=NCORES),
                og_out[:].rearrange("i p c -> p i c"))
            # keep PE warm across the collective wait
            for _ in range(8):
                d_ps = outps.tile([128, 128], f32, tag="warm")
                nc.tensor.matmul(d_ps[:], ident_b[:], ident_b[:],
                                 start=True, stop=True)
            # oh_sb[e, h*128 + i*16 + r] = ogs_sb[e, i*256 + h*16 + r]
            nc.vector.tensor_copy(
                oh_sb[:].rearrange("p (h i r) -> p h i r", h=H, i=NCORES),
                ogs_sb[:].rearrange("p (i h r) -> p h i r", i=NCORES, h=H))
            out_ps = outps.tile([RALL, DSH], f32, tag="out")
            for h in range(H):
                nc.tensor.matmul(out_ps[:], oh_sb[:, h * 128:(h + 1) * 128],
                                 wo_tiles[h][:], start=(h == 0), stop=False)
            ones_r = ones16[0:1, :]
            nc.tensor.matmul(out_ps[:], ones_r[0:1, 0:RALL], bo_sb[0:1, :],
                             start=False, stop=True)
            out_sb = const.tile([RALL, DSH], f32, tag="osb")
            nc.vector.tensor_copy(out_sb[:], out_ps[:])
            nc.scalar.dma_start(out, out_sb[:])


def _finalize_pair(tc, nc, mybir, apool, tps, j, o_ps, oT_sb, ident_f):
    f32 = mybir.dt.float32
    recip = apool.tile([128, 1], f32, tag="recip")
    nc.vector.reciprocal(recip[:], o_ps[:, HD:HD + 1])
    o_sb = apool.tile([128, HD], f32, tag="osb")
    nc.vector.tensor_scalar_mul(o_sb[:], o_ps[:, 0:HD], recip[:])
    tr = tps.tile([128, 128], f32, tag="tr")
    nc.tensor.transpose(tr[:], o_sb[:], ident_f[:])
    oT_4d = oT_sb[:].rearrange("p (h b q) -> p h b q", h=H, b=BPC)
    for b2 in range(2):
        nc.vector.tensor_copy(
            oT_4d[:, :, 2 * j + b2, :],
            tr[:, b2 * ROWS:(b2 + 1) * ROWS].rearrange(
                "p (h q) -> p h q", h=H))


def _get_nc():
    if "nc" not in _CACHE:
        _CACHE["nc"] = _build()
    return _CACHE["nc"]


def kernel(x, attn_bias, cache_k, cache_v, wq, bq, wk, bk, wv, bv, wo, bo):
    import ml_dtypes
    from concourse.bass_utils import run_bass_kernel_spmd

    nc = _get_nc()
    scale = np.float32(1.0 / np.sqrt(HD))
    bf = ml_dtypes.bfloat16

    x = np.asarray(x, np.float32)
    xT_full = np.ascontiguousarray(x.reshape(RALL, DIM).T).astype(bf)
    wq_s = np.asarray(wq, np.float32) * scale          # [DIM, H, HD]
    bq_s = np.asarray(bq, np.float32) * scale          # [H, HD]
    wk_f = np.asarray(wk, np.float32)
    wv_f = np.asarray(wv, np.float32)
    bk_f = np.asarray(bk, np.float32)
    bv_f = np.asarray(bv, np.float32)
    kTh = np.ascontiguousarray(
        np.roll(np.asarray(cache_k, np.float32), -Q, axis=1)
        .transpose(0, 2, 1)).astype(bf)
    vr0 = np.roll(np.asarray(cache_v, np.float32), -Q, axis=1)
    # [B, KV, HD] -> [B, 128, KV/128, HD+1]: per-partition-contiguous runs,
    # last column = 1.0 so the o-matmul accumulates softmax denominators
    vrh4 = vr0.reshape(B, KV // 128, 128, HD).transpose(0, 2, 1, 3)
    vrh = np.ones((B, 128, KV // 128, HD + 1), np.float32)
    vrh[..., :HD] = vrh4
    vrh = np.ascontiguousarray(vrh).astype(bf)
    # bias -> [pair, p, c, (n t r)] with kv = c*2048 + n*512 + t*128 + p
    ab = np.asarray(attn_bias, np.float32).reshape(B // 2, 2, ROWS, KV)
    abP = ab.transpose(0, 3, 1, 2).reshape(B // 2, KV, 2 * ROWS)
    biasP = np.ascontiguousarray(
        abP.reshape(B // 2, NCH, 4, 4, 128, 2 * ROWS)
        .transpose(0, 4, 1, 2, 3, 5)
        .reshape(B // 2, 128, NCH, KCH)).astype(bf)
    wo_f = np.asarray(wo, np.float32).reshape(H * HD, DIM)
    bo_f = np.asarray(bo, np.float32)

    selm = np.eye(RALL, dtype=np.float32).astype(bf)   # [RALL, RALL]
    in_maps = []
    for c in range(NCORES):
        wproj = np.concatenate(
            [wq_s[:, 2 * c:2 * c + 2, :].reshape(DIM, HPC * HD),
             wk_f, wv_f], axis=1)
        bproj = np.concatenate(
            [bq_s[2 * c:2 * c + 2].reshape(HPC * HD), bk_f, bv_f])
        in_maps.append({
            "xT": xT_full,
            "wproj": np.ascontiguousarray(wproj).astype(bf),
            "bproj": np.ascontiguousarray(bproj.reshape(1, PW)).astype(bf),
            "kT": np.ascontiguousarray(kTh[c * BPC:(c + 1) * BPC]),
            "vv": np.ascontiguousarray(vrh[c * BPC:(c + 1) * BPC]),
            "bias": np.ascontiguousarray(biasP[NPAIR * c:NPAIR * (c + 1)]),
            "wo": np.ascontiguousarray(
                wo_f[:, c * DSH:(c + 1) * DSH]).astype(bf),
            "bo": np.ascontiguousarray(
                bo_f[c * DSH:(c + 1) * DSH].reshape(1, DSH)).astype(bf),
            "ones": np.ones((1, RALL), bf),
            "sel": np.ascontiguousarray(selm[:, c * BQ:(c + 1) * BQ]),
        })

    res = run_bass_kernel_spmd(nc, in_maps, core_ids=list(range(NCORES)))
    _CACHE["last_result"] = res
    outs = [res.results[c]["out"] for c in range(NCORES)]
    full = np.concatenate(outs, axis=1)                # [128, DIM]
    return full.reshape(B, Q, DIM).astype(np.float32)
